# revision 12
# baseline (speedup 1.0000x reference)
"""FECAM layer Trainium2 kernel — v2: transpose-free, all-bf16 matmuls.

Reference (per batch element b, X = x[b] in R^{512x512}, layout [l, c]):
    freq = DCT-II(X^T along l)   [c, k]
    sd   = LN_k(freq)*gamma+beta
    h    = relu(sd @ W1^T); fw = sigmoid(h @ W2^T)
    fw   = LN_k(fw)*gamma+beta
    out  = X .* fw^T             [l, c]

Key restructuring vs the v1 (f32r + PE transposes) kernel:
  * All matmuls bf16 (1 cyc/row on PE, same rate as f32r, but halves SBUF/
    DMA and unlocks DVE 2x/4x modes). End-to-end numerics emulated on host:
    rel err ~9e-3 vs the 2e-2 gate.
  * DCT computed directly in [k, c] orientation (lhsT = DCT basis, rhs = x)
    so fc1 needs NO transpose; fc1's [h, c] output feeds fc2 as the
    stationary operand, again transpose-free; fc2's [c, k] output is natural
    for LN2; the final elementwise multiply uses HOST-pretransposed
    x^T*gamma and the output is returned in [c, l] layout which the host
    transposes back. Zero PE transposes (v1 spent ~55us on them + evicts).
  * DCT even/odd cosine symmetry: host folds x into X^+/X^- halves, halving
    DCT PE work (8192 -> 4096 cyc/batch). The even/odd k-permutation is
    folded into w1's columns.
  * LN1 stats are computed ON HOST via the DCT Gram identity
    D^T D = 2*ones + 2N*I  =>  mean/var of freq come from column sums of X
    (O(C^2) per batch, no device work). rstd1 is folded into x before the
    fold; mu1*rstd1 is uploaded as one [1, C] row per batch and partition-
    broadcast on the (otherwise idle) GPSIMD engine, then a single DVE
    tensor_sub evicts DCT psum -> LN1-normalized bf16 z.
  * LN1 gamma/beta folded into w1/b1 (host). LN2 gamma folded into the
    pretransposed x^T; LN2 beta (zero for this model's inputs) would add
    one extra DVE pass per tile (general path kept behind a flag).
  * Sigmoid as a single ACT op (sigmoid_and_others table, no table swaps);
    LN2 rstd via rsqrt bit-trick + 2 Newton steps on DVE (no ln/exp table).
  * Engine budget/batch: PE 36.9k cyc (DCT 4096 + fc1 16384 + fc2 16384),
    DVE ~7.4k, ACT ~8.8k, Pool ~1k => PE-bound at ~15.4us/batch, 16
    batches/core => ~246us + ramp vs 399us for v1.
"""

import sys

if "/opt/trn_rl_repo" not in sys.path:
    sys.path.insert(0, "/opt/trn_rl_repo")

import numpy as np

P = 128
C = 512           # channels == seq len == dct size
H = 1024          # hidden
LT = 2            # folded-l tiles (256 = 2*128)
KT = C // P       # 4 k-tiles
HT = H // P       # 8 h-tiles
CT = C // P       # 4 c-tiles
EPS = 1e-6
N_CORES = 8
B_FULL = 128
MAGIC = 0x5F3759DF

_NC_CACHE: dict = {}


def _build(nb: int, with_beta: bool):
    import concourse.bass as bass
    from concourse import bacc
    import concourse.mybir as mybir
    from concourse.tile import TileContext

    f32 = mybir.dt.float32
    bf16 = mybir.dt.bfloat16
    i32 = mybir.dt.int32
    Relu = mybir.ActivationFunctionType.Relu
    Sigmoid = mybir.ActivationFunctionType.Sigmoid
    mult = mybir.AluOpType.mult
    add = mybir.AluOpType.add
    sub = mybir.AluOpType.subtract
    shr = mybir.AluOpType.logical_shift_right

    nc = bacc.Bacc()
    xpm_d = nc.declare_dram_parameter("xpm", [nb, C, C], bf16, isOutput=False)
    xtg_d = nc.declare_dram_parameter("xtg", [nb, C, C], bf16, isOutput=False)
    mur_d = nc.declare_dram_parameter("mur", [nb, 1, C], f32, isOutput=False)
    df_d = nc.declare_dram_parameter("dfold", [2 * P, C], bf16, isOutput=False)
    w1t_d = nc.declare_dram_parameter("w1t", [C, H], bf16, isOutput=False)
    b1_d = nc.declare_dram_parameter("b1", [H], f32, isOutput=False)
    w2t_d = nc.declare_dram_parameter("w2t", [H, C], bf16, isOutput=False)
    if with_beta:
        xtb_d = nc.declare_dram_parameter("xtb", [nb, C, C], bf16, isOutput=False)
    out_d = nc.declare_dram_parameter("out", [nb, C, C], bf16, isOutput=True)

    with TileContext(nc) as tc, \
            tc.tile_pool(name="consts", bufs=1) as consts, \
            tc.tile_pool(name="xin", bufs=3) as xin, \
            tc.tile_pool(name="xtgp", bufs=3) as xtgp, \
            tc.tile_pool(name="murp", bufs=2) as murp, \
            tc.tile_pool(name="zp", bufs=2) as zp, \
            tc.tile_pool(name="hp", bufs=2) as hp, \
            tc.tile_pool(name="fwp", bufs=2) as fwp, \
            tc.tile_pool(name="resp", bufs=2) as resp, \
            tc.tile_pool(name="small", bufs=8) as small, \
            tc.tile_pool(name="ps_dct", bufs=3, space="PSUM") as ps_dct, \
            tc.tile_pool(name="ps_fc1", bufs=3, space="PSUM") as ps_fc1, \
            tc.tile_pool(name="ps_fc2", bufs=2, space="PSUM") as ps_fc2:

        # single ACT table covering Sigmoid/Relu/Identity/Copy: pre-seed so
        # the availability pass never inserts another load
        from concourse.hw_specs import get_activation_tables
        set_names = list(get_activation_tables(nc.m.arch))
        nc.scalar.add_instruction(mybir.InstLoadActFuncSet(
            name=nc.get_next_instruction_name(),
            act_func_set_id=set_names.index("sigmoid_and_others"),
            ins=[], outs=[]))

        df_sb = consts.tile([P, LT, C], bf16)
        w1t_sb = consts.tile([P, KT, H], bf16)
        w2t_sb = consts.tile([P, HT, C], bf16)
        b1_sb = consts.tile([P, HT], f32)
        magic_sb = consts.tile([P, CT], i32)
        nc.vector.memset(magic_sb, MAGIC)

        st: dict = {}   # per-batch live tiles

        def emit_load(b, first=False):
            xpm = xin.tile([P, 4, C], bf16, tag="xpm")
            murow = murp.tile([1, C], f32, tag="murow")
            if first:
                # dfold + x(0) on the sync queue so DCT(0) starts asap;
                # weights (needed ~15us later) go on the ACT hwdge queue so
                # their multi-us DIRECT2D triggers don't delay x(0)
                nc.sync.dma_start(out=df_sb,
                                  in_=df_d.rearrange("(t p) c -> p t c", p=P))
                nc.sync.dma_start(out=xpm,
                                  in_=xpm_d[b].rearrange("(g p) c -> p g c", p=P))
                nc.sync.dma_start(out=murow, in_=mur_d[b])
                nc.scalar.dma_start(out=w1t_sb,
                                    in_=w1t_d.rearrange("(t p) h -> p t h", p=P))
                nc.scalar.dma_start(out=b1_sb,
                                    in_=b1_d.rearrange("(t p) -> p t", p=P))
                nc.scalar.dma_start(out=w2t_sb,
                                    in_=w2t_d.rearrange("(t p) c -> p t c", p=P))
            else:
                nc.sync.dma_start(out=murow, in_=mur_d[b])
                nc.sync.dma_start(out=xpm,
                                  in_=xpm_d[b].rearrange("(g p) c -> p g c", p=P))
            xtg = xtgp.tile([P, CT, C], bf16, tag="xtg")
            # second hwdge queue (ACT) keeps x/out traffic unqueued behind it
            nc.scalar.dma_start(out=xtg,
                                in_=xtg_d[b].rearrange("(t p) l -> p t l", p=P))
            st[b] = {"xpm": xpm, "murow": murow, "xtg": xtg}
            if with_beta:
                xtb = xtgp.tile([P, CT, C], bf16, tag="xtb")
                nc.sync.dma_start(out=xtb,
                                  in_=xtb_d[b].rearrange("(t p) l -> p t l", p=P))
                st[b]["xtb"] = xtb

        def emit_mur(b):
            mur = murp.tile([P, C], f32, tag="mur")
            nc.gpsimd.partition_broadcast(mur, st[b]["murow"])
            st[b]["mur"] = mur

        def emit_dct(b, s):
            # doubly-folded DCT. k-slices (via w1 column permutation):
            #   s=0: k=0 mod 4  <- X++ against D[0::4] (1 matmul)
            #   s=1: k=2 mod 4  <- X+- against D[2::4] (1 matmul)
            #   s=2/3: odd k    <- X-  against D[1::2] (2 matmuls each)
            if s == 0:
                st[b]["z"] = zp.tile([P, KT, C], bf16, tag="z", name="z")
            xpm = st[b]["xpm"]
            z = st[b]["z"]
            pf = ps_dct.tile([P, C], mybir.dt.float32, tag="pf")
            if s < 2:
                nc.tensor.matmul(pf, lhsT=df_sb[:, 0, s * P:(s + 1) * P],
                                 rhs=xpm[:, s, :], start=True, stop=True)
            else:
                for lt in range(LT):
                    nc.tensor.matmul(
                        pf,
                        lhsT=df_sb[:, lt, s * P:(s + 1) * P],
                        rhs=xpm[:, lt + 2, :],
                        start=(lt == 0),
                        stop=(lt == LT - 1),
                    )
            nc.vector.tensor_sub(out=z[:, s, :], in0=pf, in1=st[b]["mur"])
            if s == KT - 1:
                del st[b]["xpm"], st[b]["murow"]

        def emit_fc1(b, mh):
            if mh == 0:
                st[b]["hT"] = hp.tile([P, HT, C], bf16, tag="hT", name="hT")
            z = st[b]["z"]
            hT = st[b]["hT"]
            ph = ps_fc1.tile([P, C], mybir.dt.float32, tag="ph")
            for kt in range(KT):
                nc.tensor.matmul(
                    ph,
                    lhsT=w1t_sb[:, kt, mh * P:(mh + 1) * P],
                    rhs=z[:, kt, :],
                    start=(kt == 0),
                    stop=(kt == KT - 1),
                )
            nc.scalar.activation(out=hT[:, mh, :], in_=ph, func=Relu,
                                 bias=b1_sb[:, mh:mh + 1], scale=1.0)
            if mh == HT - 1:
                del st[b]["z"]

        def emit_fc2(b, mc):
            if mc == 0:
                st[b]["fw"] = fwp.tile([P, CT, C], bf16, tag="fw", name="fw")
                st[b]["mv"] = small.tile([P, CT, 2], mybir.dt.float32, tag="mv", name="mv")
            hT = st[b]["hT"]
            fw = st[b]["fw"]
            pw = ps_fc2.tile([P, C], mybir.dt.float32, tag="pw")
            for ht in range(HT):
                nc.tensor.matmul(
                    pw,
                    lhsT=hT[:, ht, mc * P:(mc + 1) * P],
                    rhs=w2t_sb[:, ht, :],
                    start=(ht == 0),
                    stop=(ht == HT - 1),
                )
            nc.scalar.activation(out=fw[:, mc, :], in_=pw, func=Sigmoid,
                                 bias=0.0, scale=1.0)
            stats = small.tile([P, 6], mybir.dt.float32, tag="stats")
            nc.vector.bn_stats(out=stats, in_=fw[:, mc, :])
            nc.vector.bn_aggr(out=st[b]["mv"][:, mc, :], in_=stats)
            if mc == CT - 1:
                del st[b]["hT"]

        def emit_ln2_half(b, half):
            # LN2 apply + final multiply for c-tiles (2*half, 2*half+1),
            # emitted right after their fc2 stats so the tail of the last
            # batch overlaps the remaining fc2 matmuls.
            f32_ = mybir.dt.float32
            i32_ = mybir.dt.int32
            mv = st[b]["mv"]
            if half == 0:
                st[b]["u"] = small.tile([P, CT], f32_, tag="u", name="u")
                st[b]["y"] = small.tile([P, CT], f32_, tag="y", name="y")
                st[b]["t"] = small.tile([P, CT], f32_, tag="t", name="t")
                st[b]["res"] = resp.tile([P, CT, C], bf16, tag="res",
                                         name="res")
            u, y, t, res = (st[b][k] for k in ("u", "y", "t", "res"))
            sl = slice(2 * half, 2 * half + 2)
            # rstd2 = rsqrt(var + eps): bit-trick seed + 2 Newton steps on
            # DVE (no ln/exp act table needed)
            nc.vector.tensor_scalar_add(out=u[:, sl], in0=mv[:, sl, 1],
                                        scalar1=EPS)
            nc.vector.tensor_scalar(out=y[:, sl].bitcast(i32_),
                                    in0=u[:, sl].bitcast(i32_),
                                    scalar1=1, scalar2=None,
                                    op0=mybir.AluOpType.logical_shift_right)
            nc.vector.tensor_tensor(out=y[:, sl].bitcast(i32_),
                                    in0=magic_sb[:, sl],
                                    in1=y[:, sl].bitcast(i32_),
                                    op=mybir.AluOpType.subtract)
            for _ in range(2):
                nc.vector.tensor_mul(out=t[:, sl], in0=u[:, sl], in1=y[:, sl])
                nc.vector.tensor_mul(out=t[:, sl], in0=t[:, sl], in1=y[:, sl])
                nc.vector.tensor_scalar(out=t[:, sl], in0=t[:, sl],
                                        scalar1=-0.5, scalar2=1.5,
                                        op0=mybir.AluOpType.mult,
                                        op1=mybir.AluOpType.add)
                nc.vector.tensor_mul(out=y[:, sl], in0=y[:, sl], in1=t[:, sl])
            fw = st[b]["fw"]
            xtg = st[b]["xtg"]
            for mc in range(2 * half, 2 * half + 2):
                # res = ((fw - mu2) * rstd2) * xtg
                # tensor_scalar runs in 4x DVE mode, tensor_tensor in 2x
                # (scalar_tensor_tensor would fuse but has no fast modes)
                nc.vector.tensor_scalar(out=res[:, mc, :], in0=fw[:, mc, :],
                                        scalar1=mv[:, mc, 0:1],
                                        scalar2=y[:, mc:mc + 1],
                                        op0=mybir.AluOpType.subtract,
                                        op1=mybir.AluOpType.mult)
                nc.vector.tensor_mul(out=res[:, mc, :], in0=res[:, mc, :],
                                     in1=xtg[:, mc, :])
                if with_beta:
                    nc.vector.tensor_add(out=res[:, mc, :],
                                         in0=res[:, mc, :],
                                         in1=st[b]["xtb"][:, mc, :])
            h0 = 2 * half * P
            nc.sync.dma_start(
                out=out_d[b, h0:h0 + 2 * P, :].rearrange(
                    "(t p) l -> p t l", p=P),
                in_=res[:, sl, :])
            if half == 1:
                del st[b]

        # software pipeline, 1-batch skew:
        #   cycle i: fc1(i-1) | mur+DCT(i) | fc2(i-1) ln2+final(i-1)
        # fc1(i-1) leads (its z is long since ready); DCT(i) covers the
        # fc1->fc2 eviction gap so the PE never stalls on the relu evicts.
        for i in range(nb + 1):
            if i == 0:
                emit_load(0, first=True)
            if i + 1 < nb:
                emit_load(i + 1)
            if i >= 1:
                for mh in range(HT):
                    emit_fc1(i - 1, mh)
            if i < nb:
                emit_mur(i)
                for s in range(KT):
                    emit_dct(i, s)
            if i >= 1:
                for mc in range(CT):
                    emit_fc2(i - 1, mc)
                    if mc % 2 == 1:
                        emit_ln2_half(i - 1, mc // 2)

    nc.finalize()
    return nc


def get_nc(nb: int, with_beta: bool = False):
    key = (nb, with_beta)
    if key not in _NC_CACHE:
        _NC_CACHE[key] = _build(nb, with_beta)
    return _NC_CACHE[key]


def make_host_inputs(x, gamma, beta, w1, w2):
    """Host-side precompute: LN1 stats (Gram identity), DCT fold, weight
    folds, x^T for the final multiply. All O(B*C^2) passes."""
    import ml_dtypes
    bf = ml_dtypes.bfloat16

    x = np.ascontiguousarray(np.asarray(x, dtype=np.float32))
    gamma = np.asarray(gamma, dtype=np.float32)
    beta = np.asarray(beta, dtype=np.float32)
    w1 = np.asarray(w1, dtype=np.float32)
    w2 = np.asarray(w2, dtype=np.float32)

    k = np.arange(C)[:, None].astype(np.float64)
    m = np.arange(C)[None, :].astype(np.float64)
    D = 2.0 * np.cos(np.pi * k * (2.0 * m + 1.0) / (2.0 * C))    # [k, l]

    # LN1 stats from x via the DCT-II Gram identity D^T D = 2*ones + 2C*I
    xd = x.astype(np.float64)
    s = xd.sum(axis=1)                                  # [B, C] col sums
    q = np.einsum("blc,blc->bc", xd, xd, optimize=True)  # col sum-squares
    dbar = D.sum(axis=0)                                # [L]
    mu = np.einsum("l,blc->bc", dbar, xd, optimize=True) / C      # [B, C]
    var = (2.0 * s * s + 2.0 * C * q) / C - mu * mu
    rstd = 1.0 / np.sqrt(var + EPS)                     # [B, C]
    mur = (mu * rstd).astype(np.float32)[:, None, :]    # [B, 1, C]

    # fold rstd1 into x, then fold along l twice (even branch refolds):
    # rows of xpm: [X++ (128) | X+- (128) | X- (256)]
    xs = x * rstd[:, None, :].astype(np.float32)        # [B, L, C]
    xp_half = xs[:, :C // 2, :] + xs[:, :C // 2 - 1:-1, :]
    xm_half = xs[:, :C // 2, :] - xs[:, :C // 2 - 1:-1, :]
    xpp = xp_half[:, :C // 4, :] + xp_half[:, :C // 4 - 1:-1, :]
    xpm2 = xp_half[:, :C // 4, :] - xp_half[:, :C // 4 - 1:-1, :]
    xpm = np.concatenate([xpp, xpm2, xm_half], axis=1).astype(bf)  # [B, C, C]

    # doubly-folded DCT basis; cols = [k=0mod4 | k=2mod4 | odd k] lhsT slices
    Df = np.zeros((C // 2, C), np.float64)              # [l-fold, j]
    Df[:C // 4, 0:C // 4] = D[0::4, :C // 4].T          # X++ basis
    Df[:C // 4, C // 4:C // 2] = D[2::4, :C // 4].T     # X+- basis
    Df[:, C // 2:] = D[1::2, :C // 2].T                 # X- basis
    dfold = Df.astype(bf)

    # w1 with LN1 gamma folded, columns permuted to the folded k-order
    perm = np.concatenate([np.arange(0, C, 4), np.arange(2, C, 4),
                           np.arange(1, C, 2)])
    w1g = (w1 * gamma[None, :])[:, perm]
    w1t = np.ascontiguousarray(w1g.T).astype(bf)        # [k_perm, h]
    b1 = (w1 @ beta).astype(np.float32)                 # [h]
    w2t = np.ascontiguousarray(w2.T).astype(bf)         # [h, k]

    # pretransposed x with LN2 gamma folded (res = LN2(fw) * gamma * x^T)
    xt = np.ascontiguousarray(x.transpose(0, 2, 1))     # [B, C, L]
    xtg = (xt * gamma[None, None, :]).astype(bf)

    const = dict(dfold=dfold, w1t=w1t, b1=b1, w2t=w2t)
    per_batch = dict(xpm=xpm, xtg=xtg, mur=mur)
    with_beta = bool(np.any(beta != 0.0))
    if with_beta:
        per_batch["xtb"] = (xt * beta[None, None, :]).astype(bf)
    return per_batch, const, with_beta


def make_in_maps(per_batch, const):
    nb = B_FULL // N_CORES
    return [
        {**{k: v[i * nb:(i + 1) * nb] for k, v in per_batch.items()}, **const}
        for i in range(N_CORES)
    ]


def postprocess(results):
    """[n_cores] of {'out': [nb, C, L] bf16} -> full [B, L, C] fp32."""
    out_ct = np.concatenate([results[i]["out"] for i in range(N_CORES)], axis=0)
    return np.ascontiguousarray(
        out_ct.astype(np.float32).transpose(0, 2, 1))


def kernel(x, gamma, beta, w1, w2):
    import time
    from concourse.bass_utils import run_bass_kernel_spmd

    per_batch, const, with_beta = make_host_inputs(x, gamma, beta, w1, w2)
    nc = get_nc(B_FULL // N_CORES, with_beta)
    in_maps = make_in_maps(per_batch, const)
    last_err = None
    for attempt in range(3):
        try:
            r = run_bass_kernel_spmd(nc, in_maps, list(range(N_CORES)))
            return postprocess(r.results)
        except Exception as e:  # transient device wedge recovers on retry
            last_err = e
            time.sleep(5)
    raise last_err


# revision 20
# speedup vs baseline: 1.0153x; 1.0153x over previous
"""FECAM layer Trainium2 kernel — v2: transpose-free, all-bf16 matmuls.

Reference (per batch element b, X = x[b] in R^{512x512}, layout [l, c]):
    freq = DCT-II(X^T along l)   [c, k]
    sd   = LN_k(freq)*gamma+beta
    h    = relu(sd @ W1^T); fw = sigmoid(h @ W2^T)
    fw   = LN_k(fw)*gamma+beta
    out  = X .* fw^T             [l, c]

Key restructuring vs the v1 (f32r + PE transposes) kernel:
  * All matmuls bf16 (1 cyc/row on PE, same rate as f32r, but halves SBUF/
    DMA and unlocks DVE 2x/4x modes). End-to-end numerics emulated on host:
    rel err ~9e-3 vs the 2e-2 gate.
  * DCT computed directly in [k, c] orientation (lhsT = DCT basis, rhs = x)
    so fc1 needs NO transpose; fc1's [h, c] output feeds fc2 as the
    stationary operand, again transpose-free; fc2's [c, k] output is natural
    for LN2; the final elementwise multiply uses HOST-pretransposed
    x^T*gamma and the output is returned in [c, l] layout which the host
    transposes back. Zero PE transposes (v1 spent ~55us on them + evicts).
  * DCT even/odd cosine symmetry: host folds x into X^+/X^- halves, halving
    DCT PE work (8192 -> 4096 cyc/batch). The even/odd k-permutation is
    folded into w1's columns.
  * LN1 stats are computed ON HOST via the DCT Gram identity
    D^T D = 2*ones + 2N*I  =>  mean/var of freq come from column sums of X
    (O(C^2) per batch, no device work). rstd1 is folded into x before the
    fold; mu1*rstd1 is uploaded as one [1, C] row per batch and partition-
    broadcast on the (otherwise idle) GPSIMD engine, then a single DVE
    tensor_sub evicts DCT psum -> LN1-normalized bf16 z.
  * LN1 gamma/beta folded into w1/b1 (host). LN2 gamma folded into the
    pretransposed x^T; LN2 beta (zero for this model's inputs) would add
    one extra DVE pass per tile (general path kept behind a flag).
  * Sigmoid as a single ACT op (sigmoid_and_others table, no table swaps);
    LN2 rstd via rsqrt bit-trick + 2 Newton steps on DVE (no ln/exp table).
  * Engine budget/batch: PE 36.9k cyc (DCT 4096 + fc1 16384 + fc2 16384),
    DVE ~7.4k, ACT ~8.8k, Pool ~1k => PE-bound at ~15.4us/batch, 16
    batches/core => ~246us + ramp vs 399us for v1.
"""

import sys

if "/opt/trn_rl_repo" not in sys.path:
    sys.path.insert(0, "/opt/trn_rl_repo")

import numpy as np

P = 128
C = 512           # channels == seq len == dct size
H = 1024          # hidden
LT = 2            # folded-l tiles (256 = 2*128)
KT = C // P       # 4 k-tiles
HT = H // P       # 8 h-tiles
CT = C // P       # 4 c-tiles
EPS = 1e-6
N_CORES = 8
B_FULL = 128
MAGIC = 0x5F3759DF

_NC_CACHE: dict = {}


def _build(nb: int, with_beta: bool):
    import concourse.bass as bass
    from concourse import bacc
    import concourse.mybir as mybir
    from concourse.tile import TileContext

    f32 = mybir.dt.float32
    bf16 = mybir.dt.bfloat16
    i32 = mybir.dt.int32
    Relu = mybir.ActivationFunctionType.Relu
    Sigmoid = mybir.ActivationFunctionType.Sigmoid
    mult = mybir.AluOpType.mult
    add = mybir.AluOpType.add
    sub = mybir.AluOpType.subtract
    shr = mybir.AluOpType.logical_shift_right

    # all tensors stored partition-major in DRAM (host pre-shuffles) so
    # every DMA line is one 2-8KB contiguous chunk per partition — the
    # row-major rearrange patterns produced 1KB packets and capped the
    # DMA queues at ~20GB/s each
    nc = bacc.Bacc()
    xpm_d = nc.declare_dram_parameter("xpm", [nb, P, 4, C], bf16, isOutput=False)
    xtg_d = nc.declare_dram_parameter("xtg", [nb, P, CT, C], bf16, isOutput=False)
    mur_d = nc.declare_dram_parameter("mur", [nb, 1, C], f32, isOutput=False)
    df_d = nc.declare_dram_parameter("dfold", [P, LT, C], bf16, isOutput=False)
    w1t_d = nc.declare_dram_parameter("w1t", [P, KT, H], bf16, isOutput=False)
    b1_d = nc.declare_dram_parameter("b1", [P, HT], f32, isOutput=False)
    w2t_d = nc.declare_dram_parameter("w2t", [P, HT, C], bf16, isOutput=False)
    if with_beta:
        xtb_d = nc.declare_dram_parameter("xtb", [nb, P, CT, C], bf16,
                                          isOutput=False)
    out_d = nc.declare_dram_parameter("out", [nb, P, CT, C], bf16, isOutput=True)

    with TileContext(nc) as tc, \
            tc.tile_pool(name="consts", bufs=1) as consts, \
            tc.tile_pool(name="xin", bufs=3) as xin, \
            tc.tile_pool(name="xtgp", bufs=3) as xtgp, \
            tc.tile_pool(name="murp", bufs=2) as murp, \
            tc.tile_pool(name="zp", bufs=2) as zp, \
            tc.tile_pool(name="hp", bufs=2) as hp, \
            tc.tile_pool(name="fwp", bufs=2) as fwp, \
            tc.tile_pool(name="resp", bufs=2) as resp, \
            tc.tile_pool(name="small", bufs=8) as small, \
            tc.tile_pool(name="ps_dct", bufs=3, space="PSUM") as ps_dct, \
            tc.tile_pool(name="ps_fc1", bufs=3, space="PSUM") as ps_fc1, \
            tc.tile_pool(name="ps_fc2", bufs=2, space="PSUM") as ps_fc2:

        # single ACT table covering Sigmoid/Relu/Identity/Copy: pre-seed so
        # the availability pass never inserts another load
        from concourse.hw_specs import get_activation_tables
        set_names = list(get_activation_tables(nc.m.arch))
        nc.scalar.add_instruction(mybir.InstLoadActFuncSet(
            name=nc.get_next_instruction_name(),
            act_func_set_id=set_names.index("sigmoid_and_others"),
            ins=[], outs=[]))

        df_sb = consts.tile([P, LT, C], bf16)
        w1t_sb = consts.tile([P, KT, H], bf16)
        w2t_sb = consts.tile([P, HT, C], bf16)
        b1_sb = consts.tile([P, HT], f32)
        magic_sb = consts.tile([P, CT], i32)
        nc.vector.memset(magic_sb, MAGIC)

        st: dict = {}   # per-batch live tiles

        def emit_load(b, first=False):
            xpm = xin.tile([P, 4, C], bf16, tag="xpm")
            murow = murp.tile([1, C], f32, tag="murow")
            if first:
                # dfold + x(0) on the sync queue so DCT(0) starts asap;
                # weights (needed ~15us later) go on the ACT hwdge queue so
                # their DIRECT2D triggers/transfers don't delay x(0)
                nc.sync.dma_start(out=df_sb, in_=df_d[:])
                nc.sync.dma_start(out=xpm, in_=xpm_d[b])
                nc.sync.dma_start(out=murow, in_=mur_d[b])
                nc.scalar.dma_start(out=w1t_sb, in_=w1t_d[:])
                nc.scalar.dma_start(out=b1_sb, in_=b1_d[:])
                nc.scalar.dma_start(out=w2t_sb, in_=w2t_d[:])
            else:
                nc.sync.dma_start(out=murow, in_=mur_d[b])
                nc.sync.dma_start(out=xpm, in_=xpm_d[b])
            xtg = xtgp.tile([P, CT, C], bf16, tag="xtg")
            # second hwdge queue (ACT) keeps x/out traffic unqueued behind it
            nc.scalar.dma_start(out=xtg, in_=xtg_d[b])
            st[b] = {"xpm": xpm, "murow": murow, "xtg": xtg}
            if with_beta:
                xtb = xtgp.tile([P, CT, C], bf16, tag="xtb")
                nc.scalar.dma_start(out=xtb, in_=xtb_d[b])
                st[b]["xtb"] = xtb

        def emit_mur(b):
            mur = murp.tile([P, C], f32, tag="mur")
            nc.gpsimd.partition_broadcast(mur, st[b]["murow"])
            st[b]["mur"] = mur

        def emit_dct(b, s):
            # doubly-folded DCT. k-slices (via w1 column permutation):
            #   s=0: k=0 mod 4  <- X++ against D[0::4] (1 matmul)
            #   s=1: k=2 mod 4  <- X+- against D[2::4] (1 matmul)
            #   s=2/3: odd k    <- X-  against D[1::2] (2 matmuls each)
            if s == 0:
                st[b]["z"] = zp.tile([P, KT, C], bf16, tag="z", name="z")
            xpm = st[b]["xpm"]
            z = st[b]["z"]
            pf = ps_dct.tile([P, C], mybir.dt.float32, tag="pf")
            if s < 2:
                nc.tensor.matmul(pf, lhsT=df_sb[:, 0, s * P:(s + 1) * P],
                                 rhs=xpm[:, s, :], start=True, stop=True)
            else:
                for lt in range(LT):
                    nc.tensor.matmul(
                        pf,
                        lhsT=df_sb[:, lt, s * P:(s + 1) * P],
                        rhs=xpm[:, lt + 2, :],
                        start=(lt == 0),
                        stop=(lt == LT - 1),
                    )
            nc.vector.tensor_sub(out=z[:, s, :], in0=pf, in1=st[b]["mur"])
            if s == KT - 1:
                del st[b]["xpm"], st[b]["murow"]

        def emit_fc1(b, mh):
            if mh == 0:
                st[b]["hT"] = hp.tile([P, HT, C], bf16, tag="hT", name="hT")
            z = st[b]["z"]
            hT = st[b]["hT"]
            ph = ps_fc1.tile([P, C], mybir.dt.float32, tag="ph")
            for kt in range(KT):
                nc.tensor.matmul(
                    ph,
                    lhsT=w1t_sb[:, kt, mh * P:(mh + 1) * P],
                    rhs=z[:, kt, :],
                    start=(kt == 0),
                    stop=(kt == KT - 1),
                )
            nc.scalar.activation(out=hT[:, mh, :], in_=ph, func=Relu,
                                 bias=b1_sb[:, mh:mh + 1], scale=1.0)
            if mh == HT - 1:
                del st[b]["z"]

        def emit_fc2(b, mc):
            if mc == 0:
                st[b]["fw"] = fwp.tile([P, CT, C], bf16, tag="fw", name="fw")
                st[b]["mv"] = small.tile([P, CT, 2], mybir.dt.float32, tag="mv", name="mv")
            hT = st[b]["hT"]
            fw = st[b]["fw"]
            pw = ps_fc2.tile([P, C], mybir.dt.float32, tag="pw")
            for ht in range(HT):
                nc.tensor.matmul(
                    pw,
                    lhsT=hT[:, ht, mc * P:(mc + 1) * P],
                    rhs=w2t_sb[:, ht, :],
                    start=(ht == 0),
                    stop=(ht == HT - 1),
                )
            nc.scalar.activation(out=fw[:, mc, :], in_=pw, func=Sigmoid,
                                 bias=0.0, scale=1.0)
            stats = small.tile([P, 6], mybir.dt.float32, tag="stats")
            nc.vector.bn_stats(out=stats, in_=fw[:, mc, :])
            nc.vector.bn_aggr(out=st[b]["mv"][:, mc, :], in_=stats)
            if mc == CT - 1:
                del st[b]["hT"]

        def emit_ln2_half(b, half):
            # LN2 apply + final multiply for c-tiles (2*half, 2*half+1),
            # emitted right after their fc2 stats so the tail of the last
            # batch overlaps the remaining fc2 matmuls.
            f32_ = mybir.dt.float32
            i32_ = mybir.dt.int32
            mv = st[b]["mv"]
            if half == 0:
                st[b]["u"] = small.tile([P, CT], f32_, tag="u", name="u")
                st[b]["y"] = small.tile([P, CT], f32_, tag="y", name="y")
                st[b]["t"] = small.tile([P, CT], f32_, tag="t", name="t")
                st[b]["res"] = resp.tile([P, CT, C], bf16, tag="res",
                                         name="res")
            u, y, t, res = (st[b][k] for k in ("u", "y", "t", "res"))
            sl = slice(2 * half, 2 * half + 2)
            # rstd2 = rsqrt(var + eps): bit-trick seed + 2 Newton steps on
            # DVE (no ln/exp act table needed)
            nc.vector.tensor_scalar_add(out=u[:, sl], in0=mv[:, sl, 1],
                                        scalar1=EPS)
            nc.vector.tensor_scalar(out=y[:, sl].bitcast(i32_),
                                    in0=u[:, sl].bitcast(i32_),
                                    scalar1=1, scalar2=None,
                                    op0=mybir.AluOpType.logical_shift_right)
            nc.vector.tensor_tensor(out=y[:, sl].bitcast(i32_),
                                    in0=magic_sb[:, sl],
                                    in1=y[:, sl].bitcast(i32_),
                                    op=mybir.AluOpType.subtract)
            for _ in range(2):
                nc.vector.tensor_mul(out=t[:, sl], in0=u[:, sl], in1=y[:, sl])
                nc.vector.tensor_mul(out=t[:, sl], in0=t[:, sl], in1=y[:, sl])
                nc.vector.tensor_scalar(out=t[:, sl], in0=t[:, sl],
                                        scalar1=-0.5, scalar2=1.5,
                                        op0=mybir.AluOpType.mult,
                                        op1=mybir.AluOpType.add)
                nc.vector.tensor_mul(out=y[:, sl], in0=y[:, sl], in1=t[:, sl])
            fw = st[b]["fw"]
            xtg = st[b]["xtg"]
            for mc in range(2 * half, 2 * half + 2):
                # res = ((fw - mu2) * rstd2) * xtg
                # tensor_scalar runs in 4x DVE mode, tensor_tensor in 2x
                # (scalar_tensor_tensor would fuse but has no fast modes)
                nc.vector.tensor_scalar(out=res[:, mc, :], in0=fw[:, mc, :],
                                        scalar1=mv[:, mc, 0:1],
                                        scalar2=y[:, mc:mc + 1],
                                        op0=mybir.AluOpType.subtract,
                                        op1=mybir.AluOpType.mult)
                nc.vector.tensor_mul(out=res[:, mc, :], in0=res[:, mc, :],
                                     in1=xtg[:, mc, :])
                if with_beta:
                    nc.vector.tensor_add(out=res[:, mc, :],
                                         in0=res[:, mc, :],
                                         in1=st[b]["xtb"][:, mc, :])
            nc.sync.dma_start(out=out_d[b, :, sl, :], in_=res[:, sl, :])
            if half == 1:
                del st[b]

        # software pipeline, 1-batch skew:
        #   cycle i: fc1(i-1) | mur+DCT(i) | fc2(i-1) ln2+final(i-1)
        # fc1(i-1) leads (its z is long since ready); DCT(i) covers the
        # fc1->fc2 eviction gap so the PE never stalls on the relu evicts.
        for i in range(nb + 1):
            if i == 0:
                emit_load(0, first=True)
            if i + 1 < nb:
                emit_load(i + 1)
            if i >= 1:
                for mh in range(HT):
                    emit_fc1(i - 1, mh)
            if i < nb:
                emit_mur(i)
                for s in range(KT):
                    emit_dct(i, s)
            if i >= 1:
                for mc in range(CT):
                    emit_fc2(i - 1, mc)
                    if mc % 2 == 1:
                        emit_ln2_half(i - 1, mc // 2)

    nc.finalize()
    return nc


def get_nc(nb: int, with_beta: bool = False):
    key = (nb, with_beta)
    if key not in _NC_CACHE:
        _NC_CACHE[key] = _build(nb, with_beta)
    return _NC_CACHE[key]


def make_host_inputs(x, gamma, beta, w1, w2):
    """Host-side precompute: LN1 stats (Gram identity), DCT fold, weight
    folds, x^T for the final multiply. All O(B*C^2) passes."""
    import ml_dtypes
    bf = ml_dtypes.bfloat16

    x = np.ascontiguousarray(np.asarray(x, dtype=np.float32))
    gamma = np.asarray(gamma, dtype=np.float32)
    beta = np.asarray(beta, dtype=np.float32)
    w1 = np.asarray(w1, dtype=np.float32)
    w2 = np.asarray(w2, dtype=np.float32)

    k = np.arange(C)[:, None].astype(np.float64)
    m = np.arange(C)[None, :].astype(np.float64)
    D = 2.0 * np.cos(np.pi * k * (2.0 * m + 1.0) / (2.0 * C))    # [k, l]

    # LN1 stats from x via the DCT-II Gram identity D^T D = 2*ones + 2C*I
    xd = x.astype(np.float64)
    s = xd.sum(axis=1)                                  # [B, C] col sums
    q = np.einsum("blc,blc->bc", xd, xd, optimize=True)  # col sum-squares
    dbar = D.sum(axis=0)                                # [L]
    mu = np.einsum("l,blc->bc", dbar, xd, optimize=True) / C      # [B, C]
    var = (2.0 * s * s + 2.0 * C * q) / C - mu * mu
    rstd = 1.0 / np.sqrt(var + EPS)                     # [B, C]
    mur = (mu * rstd).astype(np.float32)[:, None, :]    # [B, 1, C]

    # fold rstd1 into x, then fold along l twice (even branch refolds):
    # rows of xpm: [X++ (128) | X+- (128) | X- (256)]
    xs = x * rstd[:, None, :].astype(np.float32)        # [B, L, C]
    xp_half = xs[:, :C // 2, :] + xs[:, :C // 2 - 1:-1, :]
    xm_half = xs[:, :C // 2, :] - xs[:, :C // 2 - 1:-1, :]
    xpp = xp_half[:, :C // 4, :] + xp_half[:, :C // 4 - 1:-1, :]
    xpm2 = xp_half[:, :C // 4, :] - xp_half[:, :C // 4 - 1:-1, :]
    xpm = np.concatenate([xpp, xpm2, xm_half], axis=1)          # [B, C, C]
    # partition-major DRAM layout: [B, P, 4, C], contiguous 4KB/partition
    xpm = np.ascontiguousarray(
        xpm.reshape(-1, 4, P, C).transpose(0, 2, 1, 3)).astype(bf)

    # doubly-folded DCT basis; cols = [k=0mod4 | k=2mod4 | odd k] lhsT slices
    Df = np.zeros((C // 2, C), np.float64)              # [l-fold, j]
    Df[:C // 4, 0:C // 4] = D[0::4, :C // 4].T          # X++ basis
    Df[:C // 4, C // 4:C // 2] = D[2::4, :C // 4].T     # X+- basis
    Df[:, C // 2:] = D[1::2, :C // 2].T                 # X- basis
    dfold = np.ascontiguousarray(
        Df.reshape(LT, P, C).transpose(1, 0, 2)).astype(bf)     # [P, LT, C]

    # w1 with LN1 gamma folded, columns permuted to the folded k-order
    perm = np.concatenate([np.arange(0, C, 4), np.arange(2, C, 4),
                           np.arange(1, C, 2)])
    w1g = (w1 * gamma[None, :])[:, perm]
    w1t = np.ascontiguousarray(
        w1g.T.reshape(KT, P, H).transpose(1, 0, 2)).astype(bf)  # [P, KT, H]
    b1 = np.ascontiguousarray(
        (w1 @ beta).astype(np.float32).reshape(HT, P).T)        # [P, HT]
    w2t = np.ascontiguousarray(
        w2.T.reshape(HT, P, C).transpose(1, 0, 2)).astype(bf)   # [P, HT, C]

    # pretransposed x with LN2 gamma folded (res = LN2(fw) * gamma * x^T)
    xt = x.transpose(0, 2, 1)                           # [B, C, L]
    xtg = np.ascontiguousarray(
        (xt * gamma[None, None, :]).reshape(-1, CT, P, C)
        .transpose(0, 2, 1, 3)).astype(bf)              # [B, P, CT, C]

    const = dict(dfold=dfold, w1t=w1t, b1=b1, w2t=w2t)
    per_batch = dict(xpm=xpm, xtg=xtg, mur=mur)
    with_beta = bool(np.any(beta != 0.0))
    if with_beta:
        per_batch["xtb"] = np.ascontiguousarray(
            (xt * beta[None, None, :]).reshape(-1, CT, P, C)
            .transpose(0, 2, 1, 3)).astype(bf)
    return per_batch, const, with_beta


def make_in_maps(per_batch, const):
    nb = B_FULL // N_CORES
    return [
        {**{k: v[i * nb:(i + 1) * nb] for k, v in per_batch.items()}, **const}
        for i in range(N_CORES)
    ]


def postprocess(results):
    """[n_cores] of {'out': [nb, P, CT, L] bf16} -> full [B, L, C] fp32."""
    out_p = np.concatenate([results[i]["out"] for i in range(N_CORES)], axis=0)
    # [B, P, CT, L] -> [B, C(=CT*P), L] -> [B, L, C]
    out_ct = out_p.astype(np.float32).transpose(0, 2, 1, 3).reshape(
        B_FULL, C, C)
    return np.ascontiguousarray(out_ct.transpose(0, 2, 1))


def kernel(x, gamma, beta, w1, w2):
    import time
    from concourse.bass_utils import run_bass_kernel_spmd

    per_batch, const, with_beta = make_host_inputs(x, gamma, beta, w1, w2)
    nc = get_nc(B_FULL // N_CORES, with_beta)
    in_maps = make_in_maps(per_batch, const)
    last_err = None
    for attempt in range(3):
        try:
            r = run_bass_kernel_spmd(nc, in_maps, list(range(N_CORES)))
            return postprocess(r.results)
        except Exception as e:  # transient device wedge recovers on retry
            last_err = e
            time.sleep(5)
    raise last_err
